# revision 18
# baseline (speedup 1.0000x reference)
"""Trainium2 Bass kernel for nn_AttentionLayer (dense_transformer).

Math (reference):
    Wi = weight_norm(in_v, in_g)            # [E, C]
    h  = (x @ Wi.T + in_b + we) * s         # s = sqrt(0.5)
    scores = h @ y                          # y = conv_feats.reshape(B, E, n)
    attn = softmax(scores, -1)
    ctx  = (attn @ y.T) * sqrt(n)
    out  = (ctx @ Wo.T + out_b + x) * s

Restructured (h is only consumed by the scores matmul, so fold the in-projection
into the small per-batch matrices):
    Z_b = s * Wi.T @ y_b                    # [C, n]   (per batch)
    sb_b = s * in_b @ y_b                   # [n]
    scores = x @ Z_b + (s*we) @ y_b + sb_b
    U_b = sqrt(n)*s * y_b.T @ Wo.T          # [n, C], plus bias row s*out_b
    out = attn_aug @ U_aug + s*x            # attn augmented with a ones column

This cuts matmul FLOPs ~2.5x and keeps every matmul operand in a layout that is
either native or obtained by on-chip PE transposes of x/we tiles.

Sharding: data-parallel over batch, 4 batches per core on 8 cores.
"""

import math

import numpy as np

import concourse.bass as bass
import concourse.bacc as bacc
import concourse.mybir as mybir
import concourse.tile as tile
from concourse.masks import make_identity

# problem shapes (hardcoded per the harness contract)
B_FULL = 32
L = 1024
C = 1024
E = 1024
HW_N = 196  # H*W
NCORES = 8
NB = B_FULL // NCORES  # batches per core
T = NB * L  # tokens per core
P = 128
KO = C // P  # 8
TT = T // P  # t-tiles per core
S_HALF = float(np.float32(math.sqrt(0.5)))
U_SCALE = float(np.float32(HW_N * math.sqrt(1.0 / HW_N)) * np.float32(math.sqrt(0.5)))

F32 = mybir.dt.float32
F32R = mybir.dt.float32r

# knobs
# Matmul-operand tiles are real float32r tensors: walrus requires fp32r matmul
# operands to be *produced* rounded (compute ops round on write; DMA does not).
# PE transposes run in plain fp32 (exact); their PSUM->SBUF copies round.
MMT = F32R

YW = KO * HW_N  # 1568 = per-batch width of y / Z flat tensors
NPAD = 256  # padded score width (>=256 keeps fp32r matmuls at full rate)


def _copy(nc, eng, dst, src, scale=None):
    if eng == "v":
        if scale is None:
            nc.vector.tensor_copy(dst, src)
        else:
            nc.vector.tensor_scalar_mul(dst, src, scale)
    else:
        if scale is None:
            nc.scalar.copy(dst, src)
        else:
            nc.scalar.activation(dst, src, mybir.ActivationFunctionType.Copy,
                                 scale=scale)


def build_kernel(nc: bass.Bass, tc: tile.TileContext, io: dict):
    x_d = io["x"]
    we_d = io["we"]
    y_d = io["y"]
    in_v_d = io["in_v"]
    in_g_d = io["in_g"]
    in_b_d = io["in_b"]
    out_v_d = io["out_v"]
    out_g_d = io["out_g"]
    out_b_d = io["out_b"]
    out_d = io["out"]
    attn_d = io["attn"]

    x_tiles = x_d.rearrange("(i p) c -> i p c", p=P)
    we_tiles = we_d.rearrange("(i p) c -> i p c", p=P)
    out_tiles = out_d.rearrange("(i p) c -> i p c", p=P)
    attn_tiles = attn_d.rearrange("(i p) v -> i p v", p=P)

    from contextlib import ExitStack

    ctx = ExitStack()

    # ---------------- persistent pools ----------------
    const = ctx.enter_context(tc.tile_pool(name="const", bufs=1))
    big = ctx.enter_context(tc.tile_pool(name="big", bufs=1))

    # PSUM pools (8 banks total: 3 + 2 + 1 + 2)
    ps_tr = ctx.enter_context(tc.tile_pool(name="ps_tr", bufs=2, space="PSUM"))
    ps_s = ctx.enter_context(tc.tile_pool(name="ps_s", bufs=2, space="PSUM"))
    ps_at = ctx.enter_context(tc.tile_pool(name="ps_at", bufs=2, space="PSUM"))
    ps_o = ctx.enter_context(tc.tile_pool(name="ps_o", bufs=2, space="PSUM"))

    ident = const.tile([P, P], F32, name="ident")
    make_identity(nc, ident)

    ones_raw = const.tile([1, P], F32, name="ones_raw")
    nc.gpsimd.memset(ones_raw[:], 1.0)
    ones_col = const.tile([1, P], MMT, name="ones_col")
    nc.vector.tensor_copy(ones_col[:], ones_raw[:])

    # g/b vectors, striped per-partition to match the standard e/c tiling:
    # element (eo*128+p) lives at [p, eo]
    in_g_sb = const.tile([P, KO], F32, name="in_g_sb")
    out_g_sb = const.tile([P, KO], F32, name="out_g_sb")
    in_b_raw = const.tile([P, KO], F32, name="in_b_raw")
    in_b_sb = const.tile([P, KO], MMT, name="in_b_sb")
    nc.sync.dma_start(in_g_sb[:], in_g_d.rearrange("(eo p) -> p eo", p=P))
    nc.sync.dma_start(out_g_sb[:], out_g_d.rearrange("(eo p) -> p eo", p=P))
    nc.sync.dma_start(in_b_raw[:], in_b_d.rearrange("(eo p) -> p eo", p=P))
    nc.vector.tensor_copy(in_b_sb[:], in_b_raw[:])
    out_b_sb = const.tile([1, C], F32, name="out_b_sb")
    nc.sync.dma_start(out_b_sb[:], out_b_d[None, :])
    out_bs_sb = const.tile([1, C], MMT, name="out_bs_sb")
    nc.scalar.activation(out_bs_sb[:], out_b_sb[:],
                         mybir.ActivationFunctionType.Copy, scale=S_HALF)

    # resident per-batch tensors
    y_sb = big.tile([P, NB * YW + 64], MMT, name="y_sb")
    z_sb = big.tile([P, NB * YW + 64], MMT, name="z_sb")
    u1_sb = big.tile([P, NB, C], MMT, name="u1_sb")
    u2_sb = big.tile([68, NB, C], MMT, name="u2_sb")
    sbias_sb = big.tile([1, NB, NPAD], MMT, name="sbias_sb")

    # pad tails so 256-wide matmul reads stay in-bounds and finite
    pad_zero = const.tile([P, 64], F32, name="pad_zero")
    nc.gpsimd.memset(pad_zero[:], 0.0)
    nc.vector.tensor_copy(y_sb[:, NB * YW : NB * YW + 64], pad_zero[:])
    nc.vector.tensor_copy(z_sb[:, NB * YW : NB * YW + 64], pad_zero[:])

    def y_slice(b, eo, width=NPAD):
        off = (b * KO + eo) * HW_N
        return y_sb[:, off : off + width]

    def z_slice(b, co, width=NPAD):
        off = (b * KO + co) * HW_N
        return z_sb[:, off : off + width]

    # ---------------- phase 0: weights, Z, U, sbias ----------------
    wpool = tc.tile_pool(name="wpool", bufs=1)
    wp = wpool.__enter__()  # noqa: held below, closed explicitly

    # y: DMA raw fp32 per batch, then round to fp32r (separate tiles keep the
    # per-instruction sync-wait count small)
    for b in range(NB):
        y_raw = wp.tile([P, YW], F32, name=f"y_raw_{b}", tag="y_raw", bufs=2)
        nc.sync.dma_start(
            y_raw.rearrange("p (eo v) -> p eo v", eo=KO),
            y_d[b].rearrange("(eo p) v -> eo p v", p=P).transpose([1, 0, 2]),
        )
        nc.vector.tensor_copy(y_sb[:, b * YW : (b + 1) * YW], y_raw[:])

    in_v_tiles = in_v_d.rearrange("(eo p) c -> eo p c", p=P)
    out_v_tiles = out_v_d.rearrange("(co p) e -> co p e", p=P)

    # Wi = in_g/||in_v_row|| * in_v, rows on partitions (e), stays for Z
    wi_tiles = []
    for eo in range(KO):
        wld = wp.tile([P, C], F32, name=f"wld_{eo}", tag="wld", bufs=2)
        nc.sync.dma_start(wld[:], in_v_tiles[eo])
        sq = wp.tile([P, C], F32, name="sq_scratch", tag="sq", bufs=2)
        nrm = wp.tile([P, 1], F32, name=f"nrm_{eo}", tag="nrm", bufs=2)
        nc.scalar.activation(sq[:], wld[:], mybir.ActivationFunctionType.Square,
                             accum_out=nrm[:])
        srt = wp.tile([P, 1], F32, name=f"srt_{eo}", tag="srt", bufs=2)
        nc.scalar.activation(srt[:], nrm[:], mybir.ActivationFunctionType.Sqrt)
        rn = wp.tile([P, 1], F32, name=f"rn_{eo}", tag="rn", bufs=2)
        nc.vector.reciprocal(rn[:], srt[:])
        fac = wp.tile([P, 1], F32, name=f"fac_{eo}", tag="fac", bufs=2)
        nc.vector.tensor_mul(fac[:], rn[:], in_g_sb[:, eo : eo + 1])
        wi = wp.tile([P, C], MMT, name=f"wi_{eo}", tag="wi", bufs=KO)
        nc.vector.tensor_scalar_mul(wi[:], wld[:], fac[:])
        wi_tiles.append(wi)

    # Z_b = s * Wi.T @ y_b   -> [C, n] stored striped like y
    for b in range(NB):
        for co in range(KO):
            zps = ps_s.tile([P, NPAD], F32, name=f"zps_{b}_{co}", tag="s")
            for eo in range(KO):
                nc.tensor.matmul(
                    zps[:],
                    (wi_tiles[eo][:, co * P : (co + 1) * P]),
                    (y_slice(b, eo)),
                    start=(eo == 0),
                    stop=(eo == KO - 1),
                )
            _copy(nc, "v" if (b * KO + co) % 2 == 0 else "s",
                  z_slice(b, co, HW_N), zps[:, :HW_N], scale=S_HALF)

        # sbias_b = s * in_b @ y_b  (single-row matmul)
        sbps = ps_at.tile([1, NPAD], F32, name=f"sbps_{b}", tag="at")
        for eo in range(KO):
            nc.tensor.matmul(
                sbps[:],
                (in_b_sb[:, eo : eo + 1]),
                (y_slice(b, eo)),
                start=(eo == 0),
                stop=(eo == KO - 1),
            )
        nc.vector.tensor_scalar_mul(sbias_sb[0:1, b, :], sbps[:], S_HALF)

    # Wo rows scaled by U_SCALE * out_g/||row||, then transposed -> WoT [e, c]
    wot = wp.tile([P, KO * C], MMT, name="wot", tag="wot")
    for co in range(KO):
        wld = wp.tile([P, E], F32, name=f"wold_{co}", tag="wld", bufs=2)
        nc.sync.dma_start(wld[:], out_v_tiles[co])
        sq = wp.tile([P, E], F32, name="sq_scratch2", tag="sq", bufs=2)
        nrm = wp.tile([P, 1], F32, name=f"onrm_{co}", tag="nrm", bufs=2)
        nc.scalar.activation(sq[:], wld[:], mybir.ActivationFunctionType.Square,
                             accum_out=nrm[:])
        srt = wp.tile([P, 1], F32, name=f"osrt_{co}", tag="srt", bufs=2)
        nc.scalar.activation(srt[:], nrm[:], mybir.ActivationFunctionType.Sqrt)
        rn = wp.tile([P, 1], F32, name=f"orn_{co}", tag="rn", bufs=2)
        nc.vector.reciprocal(rn[:], srt[:])
        fac = wp.tile([P, 1], F32, name=f"ofac_{co}", tag="fac", bufs=2)
        nc.vector.tensor_scalar(fac[:], rn[:], out_g_sb[:, co : co + 1], U_SCALE,
                                mybir.AluOpType.mult, mybir.AluOpType.mult)
        wo_s = wp.tile([P, E], F32, name=f"wo_s_{co}", tag="wo_s", bufs=2)
        nc.vector.tensor_scalar_mul(wo_s[:], wld[:], fac[:])
        # transpose the 8 [128c,128e] chunks; pack 4 per PSUM bank
        for g in range(2):
            trp = ps_tr.tile([P, 4 * P], F32, name=f"wotr_{co}_{g}", tag="tr")
            for k in range(4):
                eo = g * 4 + k
                nc.tensor.matmul(
                    trp[:, k * P : (k + 1) * P],
                    wo_s[:, eo * P : (eo + 1) * P],
                    ident[:],
                    is_transpose=True,
                    start=(k == 0),
                    stop=(k == 3),
                )
            # scatter the 4 transposed chunks to WoT[:, eo*C + co*128]
            dst = wot.rearrange("p (eo c) -> p eo c", eo=KO)[
                :, g * 4 : (g + 1) * 4, co * P : (co + 1) * P
            ]
            src = trp.rearrange("p (k c) -> p k c", k=4)
            _copy(nc, "v" if (co + g) % 2 == 0 else "s", dst, src)

    # U_b = y_b.T @ WoT  -> rows v (128 + 68), plus bias row s*out_b
    for b in range(NB):
        for vc in range(2):
            vlen = P if vc == 0 else HW_N - P
            for half in range(2):
                ups = ps_o.tile([P, 512], F32, name=f"ups_{b}_{vc}_{half}", tag="o")
                for eo in range(KO):
                    nc.tensor.matmul(
                        ups[:vlen, :],
                        (y_slice(b, eo, HW_N)[:, vc * P : vc * P + vlen]),
                        (wot[:, eo * C + half * 512 : eo * C + (half + 1) * 512]),
                        start=(eo == 0),
                        stop=(eo == KO - 1),
                    )
                dst = (
                    u1_sb[:, b, half * 512 : (half + 1) * 512]
                    if vc == 0
                    else u2_sb[0 : HW_N - P, b, half * 512 : (half + 1) * 512]
                )
                _copy(nc, "v" if (b + half) % 2 == 0 else "s", dst, ups[:vlen, :])

    wpool.__exit__(None, None, None)

    # ---------------- main loop over token tiles ----------------
    mp = tc.tile_pool(name="main", bufs=1)
    main = mp.__enter__()

    for it in range(TT):
        b = it // (L // P)

        x_t = main.tile([P, C], F32, name=f"x_{it}", tag="x", bufs=3)
        nc.sync.dma_start(x_t[:], x_tiles[it])
        we_t = main.tile([P, C], F32, name=f"we_{it}", tag="we", bufs=3)
        nc.sync.dma_start(we_t[:], we_tiles[it])

        # transpose x and we tiles: AT[c, t] (x raw, we scaled by s)
        at_x = main.tile([P, KO, P], MMT, name=f"atx_{it}", tag="atx", bufs=2)
        at_we = main.tile([P, KO, P], MMT, name=f"atwe_{it}", tag="atwe", bufs=2)
        for which, src_t, at in ((0, x_t, at_x), (1, we_t, at_we)):
            for g in range(2):
                trp = ps_tr.tile([P, 4 * P], F32, name=f"tr_{it}_{which}_{g}", tag="tr")
                for k in range(4):
                    cc = g * 4 + k
                    nc.tensor.matmul(
                        trp[:, k * P : (k + 1) * P],
                        src_t[:, cc * P : (cc + 1) * P],
                        ident[:],
                        is_transpose=True,
                        start=(k == 0),
                        stop=(k == 3),
                    )
                src = trp.rearrange("p (k c) -> p k c", k=4)
                if which == 0:
                    nc.vector.tensor_copy(at[:, g * 4 : (g + 1) * 4, :], src)
                else:
                    nc.scalar.activation(
                        at[:, g * 4 : (g + 1) * 4, :], src,
                        mybir.ActivationFunctionType.Copy, scale=S_HALF)

        # scores = x@Z_b + s*we@y_b + sbias_b   [128t, 196(+pad)]
        sps = ps_s.tile([P, NPAD], F32, name=f"sps_{it}", tag="s")
        for j in range(KO):
            nc.tensor.matmul(sps[:], (at_x[:, j, :]), (z_slice(b, j)),
                             start=(j == 0), stop=False)
        for j in range(KO):
            nc.tensor.matmul(sps[:], (at_we[:, j, :]), (y_slice(b, j)),
                             start=False, stop=False)
        nc.tensor.matmul(sps[:], (ones_col[:]), (sbias_sb[0:1, b, :]),
                         start=False, stop=True)

        # softmax over v (free dim), only the first 196 columns are real
        mneg = main.tile([P, 1], F32, name=f"mneg_{it}", tag="mneg", bufs=2)
        nc.vector.reduce_max(out=mneg[:], in_=sps[:, :HW_N],
                             axis=mybir.AxisListType.X, negate=True)
        attn_u = main.tile([P, HW_N], F32, name=f"attnu_{it}", tag="attnu", bufs=2)
        ssum = main.tile([P, 1], F32, name=f"ssum_{it}", tag="ssum", bufs=2)
        nc.scalar.activation(attn_u[:], sps[:, :HW_N],
                             mybir.ActivationFunctionType.Exp,
                             bias=mneg[:], accum_out=ssum[:])
        rcp = main.tile([P, 1], F32, name=f"rcp_{it}", tag="rcp", bufs=2)
        nc.vector.reciprocal(rcp[:], ssum[:])
        attn_n = main.tile([P, HW_N], F32, name=f"attnn_{it}", tag="attnn", bufs=2)
        nc.vector.tensor_scalar_mul(attn_n[:], attn_u[:], rcp[:])
        nc.sync.dma_start(attn_tiles[it], attn_n[:])

        # transpose attn -> [v, t] (two chunks, separate banks)
        atp1 = ps_at.tile([P, P], F32, name=f"atp1_{it}", tag="at")
        nc.tensor.matmul(atp1[:],
                         attn_n[:, 0:P], ident[:],
                         is_transpose=True, start=True, stop=True)
        atp2 = ps_at.tile([HW_N - P, P], F32, name=f"atp2_{it}", tag="at")
        nc.tensor.matmul(atp2[:],
                         attn_n[:, P:HW_N], ident[:],
                         is_transpose=True, start=True, stop=True)
        at1 = main.tile([P, P], MMT, name=f"at1_{it}", tag="at1", bufs=2)
        nc.scalar.copy(at1[:], atp1[:])
        at2 = main.tile([HW_N - P, P], MMT, name=f"at2_{it}", tag="at2", bufs=2)
        nc.vector.tensor_copy(at2[:], atp2[:])

        # out = attn_aug @ U_aug + s*x
        xs = main.tile([P, C], F32, name=f"xs_{it}", tag="xs", bufs=2)
        nc.scalar.activation(xs[:], x_t[:], mybir.ActivationFunctionType.Copy,
                             scale=S_HALF)
        o_sb = main.tile([P, C], F32, name=f"osb_{it}", tag="osb", bufs=3)
        for half in range(2):
            ops = ps_o.tile([P, 512], F32, name=f"ops_{it}_{half}", tag="o")
            nc.tensor.matmul(ops[:], (at1[:]),
                             (u1_sb[:, b, half * 512 : (half + 1) * 512]),
                             start=True, stop=False)
            nc.tensor.matmul(ops[:], (at2[:]),
                             (u2_sb[:, b, half * 512 : (half + 1) * 512]),
                             start=False, stop=False)
            nc.tensor.matmul(ops[:], (ones_col[:]),
                             (out_bs_sb[:, half * 512 : (half + 1) * 512]),
                             start=False, stop=True)
            nc.vector.tensor_add(o_sb[:, half * 512 : (half + 1) * 512], ops[:],
                                 xs[:, half * 512 : (half + 1) * 512])
        nc.sync.dma_start(out_tiles[it], o_sb[:])

    mp.__exit__(None, None, None)
    ctx.close()


def build_nc():
    nc = bacc.Bacc("TRN2", target_bir_lowering=False)
    io = {}
    io["x"] = nc.dram_tensor("x", (T, C), F32, kind="ExternalInput").ap()
    io["we"] = nc.dram_tensor("we", (T, E), F32, kind="ExternalInput").ap()
    io["y"] = nc.dram_tensor("y", (NB, E, HW_N), F32, kind="ExternalInput").ap()
    io["in_v"] = nc.dram_tensor("in_v", (E, C), F32, kind="ExternalInput").ap()
    io["in_g"] = nc.dram_tensor("in_g", (E,), F32, kind="ExternalInput").ap()
    io["in_b"] = nc.dram_tensor("in_b", (E,), F32, kind="ExternalInput").ap()
    io["out_v"] = nc.dram_tensor("out_v", (C, E), F32, kind="ExternalInput").ap()
    io["out_g"] = nc.dram_tensor("out_g", (C,), F32, kind="ExternalInput").ap()
    io["out_b"] = nc.dram_tensor("out_b", (C,), F32, kind="ExternalInput").ap()
    io["out"] = nc.dram_tensor("out", (T, C), F32, kind="ExternalOutput").ap()
    io["attn"] = nc.dram_tensor("attn", (T, HW_N), F32, kind="ExternalOutput").ap()

    with tile.TileContext(nc) as tc:
        build_kernel(nc, tc, io)
    nc.compile()
    return nc


def make_in_maps(inputs):
    x = np.ascontiguousarray(np.asarray(inputs["x"], dtype=np.float32))
    we = np.ascontiguousarray(np.asarray(inputs["word_embed"], dtype=np.float32))
    cf = np.ascontiguousarray(np.asarray(inputs["conv_feats"], dtype=np.float32))
    in_v = np.ascontiguousarray(np.asarray(inputs["in_v"], dtype=np.float32))
    in_g = np.ascontiguousarray(np.asarray(inputs["in_g"], dtype=np.float32))
    in_b = np.ascontiguousarray(np.asarray(inputs["in_b"], dtype=np.float32))
    out_v = np.ascontiguousarray(np.asarray(inputs["out_v"], dtype=np.float32))
    out_g = np.ascontiguousarray(np.asarray(inputs["out_g"], dtype=np.float32))
    out_b = np.ascontiguousarray(np.asarray(inputs["out_b"], dtype=np.float32))

    in_maps = []
    for core in range(NCORES):
        b0 = core * NB
        in_maps.append(
            {
                "x": x[b0 : b0 + NB].reshape(T, C),
                "we": we[b0 : b0 + NB].reshape(T, E),
                "y": cf[b0 : b0 + NB].reshape(NB, E, HW_N),
                "in_v": in_v,
                "in_g": in_g,
                "in_b": in_b,
                "out_v": out_v,
                "out_g": out_g,
                "out_b": out_b,
            }
        )
    return in_maps


def kernel(**inputs):
    from concourse.bass_utils import run_bass_kernel_spmd

    nc = build_nc()
    in_maps = make_in_maps(inputs)
    res = run_bass_kernel_spmd(nc, in_maps, core_ids=list(range(NCORES)))
    out = np.empty((B_FULL, L, C), dtype=np.float32)
    attn = np.empty((B_FULL, L, HW_N), dtype=np.float32)
    for core in range(NCORES):
        b0 = core * NB
        out[b0 : b0 + NB] = res.results[core]["out"].reshape(NB, L, C)
        attn[b0 : b0 + NB] = res.results[core]["attn"].reshape(NB, L, HW_N)
    return out, attn
